# revision 21
# baseline (speedup 1.0000x reference)
"""InvertedReorg (depth-to-space, slice=2) Trainium2 Bass kernel.

Full input x: (32, 256, 64, 64) f32 -> output (32, 64, 128, 128) f32 with
    y[b, c, s1*64 + h, s2*64 + w] = x[b, s1*128 + s2*64 + c, h, w]
i.e. the output image is a 2x2 grid of 64x64 blocks, each block one full
input channel map. Data-parallel over batch: 4 samples per core.

All-direct schedule (measured fastest of ~12 variants): every (b, s1, s2)
channel group is one DRAM->DRAM DMA (source: 64 channel maps x 16 KiB
contiguous; dest: 64x64 rows of 256B at stride 512B), round-robined
across the three DMA rings (SP HWDGE, ACT HWDGE, gpsimd SWDGE). Each
byte crosses HBM exactly once each way with no SBUF round-trip - half
the DMA work of a load/shuffle/store pipeline, no compute, and no
inter-DMA dependencies at all. An SBUF-staged pipeline variant
(95.5 us) and a hybrid (101.8 us) both measured slower than this
(75.3 us); SWDGE aggregates the scattered 256B writes into 4 KiB
packets, and the three rings drain the 16 shared SDMA engines
concurrently.
"""

import numpy as np

_B, _CH, _H, _W = 32, 256, 64, 64
_NCORES = 8
_BPC = _B // _NCORES  # samples per core
_C = _CH // 4  # output channels
_HW = _H * _W  # 4096
_FD = 2 * _HW  # 8192 free-dim elements per partition

_cache = {}


def _split_multiwaits(nc, mybir):
    """This walrus build allows one sync-wait command per instruction.
    Tile attaches one wait per dependency, so split the extras into
    same-engine NoOps directly preceding the instruction (the engine
    blocks on each in turn - semantics unchanged)."""
    for f in nc.m.functions:
        for b in f.blocks:
            new_insts = []
            for inst in b.instructions:
                si = inst.sync_info
                if si is not None and len(si.on_wait) > 1:
                    for w in si.on_wait[:-1]:
                        new_insts.append(
                            mybir.InstNoOp(
                                name=f"I-{nc.next_id()}",
                                engine=inst.engine,
                                ins=[],
                                outs=[],
                                sync_info=mybir.SyncInfo(on_wait=[w], on_update=[]),
                            )
                        )
                    inst.sync_info = mybir.SyncInfo(
                        on_wait=[si.on_wait[-1]], on_update=list(si.on_update)
                    )
                new_insts.append(inst)
            b.instructions = new_insts


def _build():
    from concourse import bass, mybir, tile

    nc = bass.Bass()
    x = nc.declare_dram_parameter(
        "x", [_BPC, _CH, _H, _W], mybir.dt.float32, isOutput=False
    )
    y = nc.declare_dram_parameter(
        "y", [_BPC, _C, 2 * _H, 2 * _W], mybir.dt.float32, isOutput=True
    )
    # x viewed as [b, s1, s2, c, (h w)]
    xr = x.rearrange("b (s1 s2 c) h w -> b s1 s2 c (h w)", s1=2, s2=2)
    # y viewed as [b, (c s1), (h w)] -- partition p = 2c + s1 (SBUF path)
    yr = y.rearrange("b c (s hh) w -> b (c s) (hh w)", s=2)
    # y viewed as [b, s1, s2, c, hh, w] (direct path)
    y6 = y.rearrange("b c (s1 hh) (s2 w) -> b s1 s2 c hh w", s1=2, s2=2)

    # SWDGE (gpsimd) aggregates the 256B rows into 4 KiB packets and runs
    # ~223 GB/s; the HWDGE rings do ~80-95 GB/s each on this pattern.
    # Interleaved round-robin 6/5/5 measured best (75.3 us); weighted
    # splits (9:4:3, 6g-first) were equal-or-worse -- heavy SWDGE load
    # slows the HWDGE rings via the shared 16 SDMA engines.
    engines = [nc.sync, nc.scalar, nc.gpsimd]
    with tile.TileContext(nc) as tc:
        i = 0
        for b in range(_BPC):
            for s1 in range(2):
                for s2 in range(2):
                    src = xr[b, s1, s2].rearrange("c (h w) -> c h w", w=_W)
                    engines[i % 3].dma_start(out=y6[b, s1, s2], in_=src)
                    i += 1
    _split_multiwaits(nc, mybir)
    return nc


def kernel(x: np.ndarray) -> np.ndarray:
    from concourse.bass_utils import run_bass_kernel_spmd

    if "nc" not in _cache:
        _cache["nc"] = _build()
    nc = _cache["nc"]

    x = np.ascontiguousarray(np.asarray(x), dtype=np.float32)
    in_maps = [{"x": x[i * _BPC : (i + 1) * _BPC]} for i in range(_NCORES)]
    res = run_bass_kernel_spmd(nc, in_maps, list(range(_NCORES)))
    return np.concatenate([res.results[i]["y"] for i in range(_NCORES)], axis=0)
